# revision 26
# baseline (speedup 1.0000x reference)
"""Trainium2 Bass kernel for the RNN-T JointNetwork problem.

  enc = h_enc @ W_enc + b_enc            (B,T,1,J)
  dec = h_dec @ W_dec                    (B,1,U,J)
  z   = tanh(enc + dec)                  (B,T,U,J)
  out = z @ W_out + b_out                (B,T,U,V)

Shapes: B=4, T=256, U=64, D=J=V=512, fp32 in/out.

Sharding: 8 cores, data parallel over (B x T/2): core c handles batch
b = c//2 and t-half th = c%2 (128 t values). Params replicated.

The tiny enc/dec projections (0.3 of 17.5 GFLOP) are computed on the
host in fp32 and shipped as bf16; 98% of the FLOPs (z @ W_out) plus the
broadcast-add and tanh run on device:

  per row-block q (2048 rows of (t,u)):
    zpre[j,(t,u)] = dec16 bcast + enc_dup pairs   (DVE, bf16; enc is
        shipped value-duplicated [j,2t] so the innermost axis is step-1,
        which keeps the broadcast add in the DVE's packed 2x mode)
    zT[j, rows]   = tanh(zpre)                    (ACT -> persistent zT)
    per v-chunk vc: psum[v,rows] = sum_jc W_out[jc,vc].T @ zT[jc]
        (W_out chunk is the STATIONARY operand -> v on partitions,
         4 interleaved accumulation groups over one 4-bank PSUM tile)
    evict: out_sb = psum + b_out[vc] (per-partition scalar; split
        5:3 between DVE and ACT) cast to bf16 -> DMA out[v, rows]

Dummy matmuls bridge the initial tanh-paced stretch so the PE's HAM
clock-gate warms to 2.4 GHz before the dense matmul stream begins.
Host reassembles out[v, t*64+u] -> (B,T,U,V) fp32.
"""

import numpy as np

B, T, U = 4, 256, 64
D, J, V = 512, 512, 512
NCORES = 8
TH = T // 2          # t's per core = 128
R = TH * U           # rows of (t,u) per core = 8192
KC = 4               # 512/128 chunks
QN = 4               # row blocks
QR = R // QN         # 2048 rows per block
TQ = TH // QN        # 32 t's per block

_compiled = None


def _build():
    import concourse.bass as bass
    import concourse.tile as tile
    from concourse import mybir

    fp32 = mybir.dt.float32
    bf16 = mybir.dt.bfloat16
    AF = mybir.ActivationFunctionType

    nc = bass.Bass()

    # all inputs arrive pre-shuffled partition-major ([128, k*cols]) so
    # each DMA moves dense 2-4KB lines per partition
    encd_d = nc.declare_dram_parameter(
        "encd", [128, KC * 2 * TH], bf16, isOutput=False
    )
    dec16_d = nc.declare_dram_parameter(
        "dec16", [128, KC * U], bf16, isOutput=False
    )
    wout = nc.declare_dram_parameter(
        "wout", [128, KC * V], bf16, isOutput=False
    )
    bout = nc.declare_dram_parameter("bout", [128, KC], fp32, isOutput=False)
    out = nc.declare_dram_parameter("out", [V, R], bf16, isOutput=True)



    with tile.TileContext(nc) as tc:
        with (
            tc.tile_pool(name="const", bufs=1) as const,
            tc.tile_pool(name="zpre", bufs=8) as zpre_pool,
            tc.tile_pool(name="outs", bufs=6) as outs_pool,
            tc.tile_pool(name="ps", bufs=2, space="PSUM") as ps_pool,
        ):
            # ---- input DMAs, critical-first, split across the two
            # hardware-DGE queues (Sync and Scalar) ----
            encd = const.tile([128, KC * 2 * TH], bf16, tag="encd")
            nc.sync.dma_start(encd[:], encd_d[:, :])
            dec16 = const.tile([128, KC * U], bf16, tag="dec16")
            nc.scalar.dma_start(dec16[:], dec16_d[:, :])
            wout_s = const.tile([128, KC * V], bf16, tag="wout")
            nc.scalar.dma_start(wout_s[:], wout[:, :])
            bout_s = const.tile([128, KC], fp32, tag="bout")
            nc.sync.dma_start(bout_s[:], bout[:, :])
            # zeroed dummy-matmul operand: lets the PE start immediately
            # (no DMA dependency) to warm the HAM clock-gate
            dmy = const.tile([128, 512], bf16, tag="dmy")
            nc.gpsimd.memset(dmy[:], 0)

            # ---- persistent zT (moving operand of the main matmul) ----
            zt = []
            for jc in range(KC):
                t_ = const.tile([128, R], bf16, tag=f"zt{jc}")
                zt.append(t_)

            def emit_z(q, halves=1, jcs=range(KC)):
                # zpre[j, (t, u)] = dec16[j, u] + enc_dup[j, 2t..2t+1]
                hr = QR // halves
                ht = TQ // halves
                for jc in jcs:
                    for h in range(halves):
                        zp = zpre_pool.tile([128, QR], bf16, tag="zp")
                        out4 = zp[:, 0:hr].rearrange(
                            "p (t uh two) -> p t uh two",
                            t=ht, uh=U // 2, two=2,
                        )
                        e0 = jc * 2 * TH + q * 2 * TQ + h * 2 * ht
                        enc4 = (
                            encd[:, e0:e0 + 2 * ht]
                            .rearrange("p (t x two) -> p t x two", x=1, two=2)
                            .to_broadcast([128, ht, U // 2, 2])
                        )
                        dec4 = (
                            dec16[:, jc * U:(jc + 1) * U]
                            .rearrange("p (x uh two) -> p x uh two", x=1, two=2)
                            .to_broadcast([128, ht, U // 2, 2])
                        )
                        nc.vector.tensor_add(out4, dec4, enc4)
                        r0 = q * QR + h * hr
                        nc.scalar.activation(
                            zt[jc][:, r0:r0 + hr], zp[:, 0:hr], AF.Tanh
                        )

            # dummy-matmul helper: keeps the PE busy (HAM warm) while the
            # first z blocks are produced; writes a scratch psum region
            ps0 = ps_pool.tile([128, QR // 2], fp32, tag="po")

            def dummies(n):
                for _ in range(n):
                    nc.tensor.matmul(
                        ps0[:, 0:512],
                        dmy[:, 0:128],
                        dmy[:],
                        start=True,
                        stop=True,
                    )

            def emit_mm_vc(q, vc):
                po = ps_pool.tile([128, QR], fp32, tag="po")
                for jc in range(KC):
                    if q == 0 and vc == 0 and jc > 0:
                        dummies(4)
                    lhsT = wout_s[:, jc * V + vc * 128:
                                  jc * V + vc * 128 + 128]
                    for rg in range(4):
                        nc.tensor.matmul(
                            po[:, rg * 512:(rg + 1) * 512],
                            lhsT,
                            zt[jc][:, q * QR + rg * 512:
                                   q * QR + (rg + 1) * 512],
                            start=(jc == 0),
                            stop=(jc == KC - 1),
                        )
                last = (q == QN - 1) and (vc == KC - 1)
                ob = outs_pool.tile([128, QR], bf16, tag="ob")
                if not last:
                    if vc % 2 == 0:
                        nc.vector.tensor_scalar_add(
                            ob[:], po[:], bout_s[:, vc:vc + 1]
                        )
                    else:
                        nc.scalar.activation(
                            ob[:], po[:], AF.Identity,
                            bias=bout_s[:, vc:vc + 1],
                        )
                    nc.sync.dma_start(
                        out[vc * 128:(vc + 1) * 128, q * QR:(q + 1) * QR],
                        ob[:],
                    )
                else:
                    # final tile: fine-grained eviction on both engines
                    # in parallel to shorten the kernel tail
                    for rg in range(4):
                        sl = slice(rg * 512, (rg + 1) * 512)
                        if (vc + rg) % 2 == 0:
                            nc.vector.tensor_scalar_add(
                                ob[:, sl], po[:, sl], bout_s[:, vc:vc + 1]
                            )
                        else:
                            nc.scalar.activation(
                                ob[:, sl], po[:, sl], AF.Identity,
                                bias=bout_s[:, vc:vc + 1],
                            )
                        nc.sync.dma_start(
                            out[vc * 128:(vc + 1) * 128,
                                q * QR + rg * 512: q * QR + (rg + 1) * 512],
                            ob[:, sl],
                        )

            # software pipeline: z(q+1) production is interleaved per-jc
            # with block q's per-vc matmul groups, so each helper engine's
            # queue alternates tanh/add work with evictions in deadline
            # order (tanh jc0, evict vc0, tanh jc1, evict vc1, ...)
            dummies(13)
            emit_z(0, halves=2)
            for q in range(QN):
                for vc in range(KC):
                    if q + 1 < QN:
                        emit_z(q + 1, jcs=[vc])
                    emit_mm_vc(q, vc)

    _split_multi_waits(nc)
    return nc


_COMPUTE_OPS = {
    "Matmult", "Ldweights", "TensorTensor", "TensorCopy", "TensorScalarPtr",
    "Activation", "TensorReduce", "Memset", "ScalarTensorTensor",
    "TensorScalar", "DMACopy", "Drain", "EventSemaphore",
}


def _split_multi_waits(nc):
    """walrus codegen in this container allows a single sync-wait command
    per TPB compute instruction; Tile emits several.  Hoist all but one
    wait onto standalone EventSemaphore instructions placed just before
    the offending instruction (same engine, so semantics are identical).
    """
    from concourse import mybir

    ctr = [0]
    for fn in nc.m.functions:
        for blk in fn.blocks:
            insts = blk.instructions
            out = []
            for inst in insts:
                si = getattr(inst, "sync_info", None)
                ow = list(si.on_wait) if si and si.on_wait else []
                if (
                    len(ow) > 1
                    and getattr(inst, "opcode", None) in _COMPUTE_OPS
                ):
                    for w in ow[:-1]:
                        ctr[0] += 1
                        ev = mybir.InstEventSemaphore(
                            name=f"WS-{ctr[0]}-{inst.name}",
                            ins=[],
                            outs=[],
                            sync_info=mybir.SyncInfo(
                                on_wait=[w], on_update=[]
                            ),
                        )
                        ev.engine = inst.engine
                        out.append(ev)
                    inst.sync_info = mybir.SyncInfo(
                        on_wait=[ow[-1]], on_update=list(si.on_update or [])
                    )
                out.append(inst)
            blk.instructions = out


def _get_compiled():
    global _compiled
    if _compiled is None:
        _compiled = _build()
    return _compiled


def kernel(h_enc, h_dec, W_enc, b_enc, W_dec, W_out, b_out, **_):
    nc = _get_compiled()
    from concourse.bass_utils import run_bass_kernel_spmd
    import ml_dtypes

    bfl = ml_dtypes.bfloat16
    h_enc = np.asarray(h_enc, dtype=np.float32)
    h_dec = np.asarray(h_dec, dtype=np.float32)
    W_enc = np.asarray(W_enc, dtype=np.float32)
    W_dec = np.asarray(W_dec, dtype=np.float32)
    b_enc = np.asarray(b_enc, dtype=np.float32)
    def pmajor(a):
        # (512, C) -> partition-major (128, 4*C): row p holds the four
        # 128-row-chunk rows (k*128+p) concatenated, giving the SBUF
        # layout directly so the DMA moves dense per-partition lines
        c = a.shape[1]
        return np.ascontiguousarray(
            a.reshape(KC, 128, c).transpose(1, 0, 2).reshape(128, KC * c)
        )

    wout_b = pmajor(np.asarray(W_out, dtype=np.float32).astype(bfl))
    bout_cols = np.ascontiguousarray(
        np.asarray(b_out, dtype=np.float32).reshape(KC, 128).T
    )

    # host-side enc/dec projections (0.3 of 17.5 GFLOP), fp32 then bf16
    enc = h_enc[:, :, 0, :] @ W_enc + b_enc      # (B, T, J)
    dec = h_dec[:, 0, :, :] @ W_dec              # (B, U, J)

    in_maps = []
    for c in range(NCORES):
        b, th = c // 2, c % 2
        encT = enc[b, th * TH:(th + 1) * TH, :].T          # (J, TH)
        encd = pmajor(
            np.repeat(encT, 2, axis=1).astype(bfl)
        )  # each value duplicated into adjacent pairs
        dec16 = pmajor(dec[b].T.astype(bfl))
        in_maps.append(
            {
                "encd": encd,
                "dec16": dec16,
                "wout": wout_b,
                "bout": bout_cols,
            }
        )

    global _last_in_maps
    _last_in_maps = in_maps
    res = run_bass_kernel_spmd(nc, in_maps, list(range(NCORES)))

    out_full = np.empty((B, T, U, V), dtype=np.float32)
    for c in range(NCORES):
        b, th = c // 2, c % 2
        oc = np.asarray(res.results[c]["out"]).astype(np.float32)  # (512, 8192)
        out_full[b, th * TH:(th + 1) * TH] = oc.reshape(V, TH, U).transpose(
            1, 2, 0
        )
    return out_full


# revision 27
# speedup vs baseline: 1.2383x; 1.2383x over previous
"""Trainium2 Bass kernel for the RNN-T JointNetwork problem.

  enc = h_enc @ W_enc + b_enc            (B,T,1,J)
  dec = h_dec @ W_dec                    (B,1,U,J)
  z   = tanh(enc + dec)                  (B,T,U,J)
  out = z @ W_out + b_out                (B,T,U,V)

Shapes: B=4, T=256, U=64, D=J=V=512, fp32 in/out.

Sharding: 8 cores, data parallel over (B x T/2): core c handles batch
b = c//2 and t-half th = c%2 (128 t values). Params replicated.

The tiny enc/dec projections (0.3 of 17.5 GFLOP) are computed on the
host in fp32 and shipped as bf16; 98% of the FLOPs (z @ W_out) plus the
broadcast-add and tanh run on device:

  per row-block q (2048 rows of (t,u)):
    zpre[j,(t,u)] = dec16 bcast + enc_dup pairs   (DVE, bf16; enc is
        shipped value-duplicated [j,2t] so the innermost axis is step-1,
        which keeps the broadcast add in the DVE's packed 2x mode)
    zT[j, rows]   = tanh(zpre)                    (ACT -> persistent zT)
    per v-chunk vc: psum[v,rows] = sum_jc W_out[jc,vc].T @ zT[jc]
        (W_out chunk is the STATIONARY operand -> v on partitions,
         4 interleaved accumulation groups over one 4-bank PSUM tile)
    evict: out_sb = psum + b_out[vc] (per-partition scalar; split
        5:3 between DVE and ACT) cast to bf16 -> DMA out[v, rows]

Dummy matmuls bridge the initial tanh-paced stretch so the PE's HAM
clock-gate warms to 2.4 GHz before the dense matmul stream begins.
Host reassembles out[v, t*64+u] -> (B,T,U,V) fp32.
"""

import numpy as np

B, T, U = 4, 256, 64
D, J, V = 512, 512, 512
NCORES = 8
TH = T // 2          # t's per core = 128
R = TH * U           # rows of (t,u) per core = 8192
KC = 4               # 512/128 chunks
QN = 4               # row blocks
QR = R // QN         # 2048 rows per block
TQ = TH // QN        # 32 t's per block

_compiled = None


def _build():
    import concourse.bass as bass
    import concourse.tile as tile
    from concourse import mybir

    fp32 = mybir.dt.float32
    bf16 = mybir.dt.bfloat16
    AF = mybir.ActivationFunctionType

    nc = bass.Bass()

    # all inputs arrive pre-shuffled partition-major ([128, k*cols]) so
    # each DMA moves dense 2-4KB lines per partition
    encd_d = nc.declare_dram_parameter(
        "encd", [128, KC * 2 * TH], bf16, isOutput=False
    )
    dec16_d = nc.declare_dram_parameter(
        "dec16", [128, KC * U], bf16, isOutput=False
    )
    wout = nc.declare_dram_parameter(
        "wout", [128, KC * V], bf16, isOutput=False
    )
    bout = nc.declare_dram_parameter("bout", [128, KC], fp32, isOutput=False)
    out = nc.declare_dram_parameter("out", [V, R], bf16, isOutput=True)



    with tile.TileContext(nc) as tc:
        with (
            tc.tile_pool(name="const", bufs=1) as const,
            tc.tile_pool(name="zpre", bufs=8) as zpre_pool,
            tc.tile_pool(name="outs", bufs=6) as outs_pool,
            tc.tile_pool(name="ps", bufs=2, space="PSUM") as ps_pool,
        ):
            # ---- input DMAs, critical-first, split across the two
            # hardware-DGE queues (Sync and Scalar) ----
            encd = const.tile([128, KC * 2 * TH], bf16, tag="encd")
            nc.sync.dma_start(encd[:], encd_d[:, :])
            dec16 = const.tile([128, KC * U], bf16, tag="dec16")
            nc.scalar.dma_start(dec16[:], dec16_d[:, :])
            wout_s = const.tile([128, KC * V], bf16, tag="wout")
            nc.scalar.dma_start(wout_s[:], wout[:, :])
            bout_s = const.tile([128, KC], fp32, tag="bout")
            nc.sync.dma_start(bout_s[:], bout[:, :])
            # zeroed dummy-matmul operand: lets the PE start immediately
            # (no DMA dependency) to warm the HAM clock-gate
            dmy = const.tile([128, 512], bf16, tag="dmy")
            nc.gpsimd.memset(dmy[:], 0)

            # ---- persistent zT (moving operand of the main matmul) ----
            zt = []
            for jc in range(KC):
                t_ = const.tile([128, R], bf16, tag=f"zt{jc}")
                zt.append(t_)

            def emit_z(q, halves=1, jcs=range(KC)):
                # zpre[j, (t, u)] = dec16[j, u] + enc_dup[j, 2t..2t+1]
                hr = QR // halves
                ht = TQ // halves
                for jc in jcs:
                    for h in range(halves):
                        zp = zpre_pool.tile([128, QR], bf16, tag="zp")
                        out4 = zp[:, 0:hr].rearrange(
                            "p (t uh two) -> p t uh two",
                            t=ht, uh=U // 2, two=2,
                        )
                        e0 = jc * 2 * TH + q * 2 * TQ + h * 2 * ht
                        enc4 = (
                            encd[:, e0:e0 + 2 * ht]
                            .rearrange("p (t x two) -> p t x two", x=1, two=2)
                            .to_broadcast([128, ht, U // 2, 2])
                        )
                        dec4 = (
                            dec16[:, jc * U:(jc + 1) * U]
                            .rearrange("p (x uh two) -> p x uh two", x=1, two=2)
                            .to_broadcast([128, ht, U // 2, 2])
                        )
                        nc.vector.tensor_add(out4, dec4, enc4)
                        r0 = q * QR + h * hr
                        nc.scalar.activation(
                            zt[jc][:, r0:r0 + hr], zp[:, 0:hr], AF.Tanh
                        )

            # dummy-matmul helper: keeps the PE busy (HAM warm) while the
            # first z blocks are produced; writes a scratch psum region
            ps0 = ps_pool.tile([128, QR // 2], fp32, tag="po")

            def dummies(n):
                for _ in range(n):
                    nc.tensor.matmul(
                        ps0[:, 0:512],
                        dmy[:, 0:128],
                        dmy[:],
                        start=True,
                        stop=True,
                    )

            def emit_mm_vc(q, vc):
                po = ps_pool.tile([128, QR], fp32, tag="po")
                for jc in range(KC):
                    if q == 0 and vc == 0 and jc > 0:
                        dummies(5)
                    lhsT = wout_s[:, jc * V + vc * 128:
                                  jc * V + vc * 128 + 128]
                    for rg in range(4):
                        nc.tensor.matmul(
                            po[:, rg * 512:(rg + 1) * 512],
                            lhsT,
                            zt[jc][:, q * QR + rg * 512:
                                   q * QR + (rg + 1) * 512],
                            start=(jc == 0),
                            stop=(jc == KC - 1),
                        )
                last = (q == QN - 1) and (vc == KC - 1)
                ob = outs_pool.tile([128, QR], bf16, tag="ob")
                if not last:
                    if vc % 2 == 0:
                        nc.vector.tensor_scalar_add(
                            ob[:], po[:], bout_s[:, vc:vc + 1]
                        )
                    else:
                        nc.scalar.activation(
                            ob[:], po[:], AF.Identity,
                            bias=bout_s[:, vc:vc + 1],
                        )
                    nc.sync.dma_start(
                        out[vc * 128:(vc + 1) * 128, q * QR:(q + 1) * QR],
                        ob[:],
                    )
                else:
                    # final tile: fine-grained eviction on both engines
                    # in parallel to shorten the kernel tail
                    for rg in range(4):
                        sl = slice(rg * 512, (rg + 1) * 512)
                        if (vc + rg) % 2 == 0:
                            nc.vector.tensor_scalar_add(
                                ob[:, sl], po[:, sl], bout_s[:, vc:vc + 1]
                            )
                        else:
                            nc.scalar.activation(
                                ob[:, sl], po[:, sl], AF.Identity,
                                bias=bout_s[:, vc:vc + 1],
                            )
                        nc.sync.dma_start(
                            out[vc * 128:(vc + 1) * 128,
                                q * QR + rg * 512: q * QR + (rg + 1) * 512],
                            ob[:, sl],
                        )

            # software pipeline: z(q+1) production is interleaved per-jc
            # with block q's per-vc matmul groups, so each helper engine's
            # queue alternates tanh/add work with evictions in deadline
            # order (tanh jc0, evict vc0, tanh jc1, evict vc1, ...)
            dummies(16)
            emit_z(0, halves=4)
            for q in range(QN):
                for vc in range(KC):
                    if q + 1 < QN:
                        emit_z(q + 1, jcs=[vc])
                    emit_mm_vc(q, vc)

    _split_multi_waits(nc)
    return nc


_COMPUTE_OPS = {
    "Matmult", "Ldweights", "TensorTensor", "TensorCopy", "TensorScalarPtr",
    "Activation", "TensorReduce", "Memset", "ScalarTensorTensor",
    "TensorScalar", "DMACopy", "Drain", "EventSemaphore",
}


def _split_multi_waits(nc):
    """walrus codegen in this container allows a single sync-wait command
    per TPB compute instruction; Tile emits several.  Hoist all but one
    wait onto standalone EventSemaphore instructions placed just before
    the offending instruction (same engine, so semantics are identical).
    """
    from concourse import mybir

    ctr = [0]
    for fn in nc.m.functions:
        for blk in fn.blocks:
            insts = blk.instructions
            out = []
            for inst in insts:
                si = getattr(inst, "sync_info", None)
                ow = list(si.on_wait) if si and si.on_wait else []
                if (
                    len(ow) > 1
                    and getattr(inst, "opcode", None) in _COMPUTE_OPS
                ):
                    for w in ow[:-1]:
                        ctr[0] += 1
                        ev = mybir.InstEventSemaphore(
                            name=f"WS-{ctr[0]}-{inst.name}",
                            ins=[],
                            outs=[],
                            sync_info=mybir.SyncInfo(
                                on_wait=[w], on_update=[]
                            ),
                        )
                        ev.engine = inst.engine
                        out.append(ev)
                    inst.sync_info = mybir.SyncInfo(
                        on_wait=[ow[-1]], on_update=list(si.on_update or [])
                    )
                out.append(inst)
            blk.instructions = out


def _get_compiled():
    global _compiled
    if _compiled is None:
        _compiled = _build()
    return _compiled


def kernel(h_enc, h_dec, W_enc, b_enc, W_dec, W_out, b_out, **_):
    nc = _get_compiled()
    from concourse.bass_utils import run_bass_kernel_spmd
    import ml_dtypes

    bfl = ml_dtypes.bfloat16
    h_enc = np.asarray(h_enc, dtype=np.float32)
    h_dec = np.asarray(h_dec, dtype=np.float32)
    W_enc = np.asarray(W_enc, dtype=np.float32)
    W_dec = np.asarray(W_dec, dtype=np.float32)
    b_enc = np.asarray(b_enc, dtype=np.float32)
    def pmajor(a):
        # (512, C) -> partition-major (128, 4*C): row p holds the four
        # 128-row-chunk rows (k*128+p) concatenated, giving the SBUF
        # layout directly so the DMA moves dense per-partition lines
        c = a.shape[1]
        return np.ascontiguousarray(
            a.reshape(KC, 128, c).transpose(1, 0, 2).reshape(128, KC * c)
        )

    wout_b = pmajor(np.asarray(W_out, dtype=np.float32).astype(bfl))
    bout_cols = np.ascontiguousarray(
        np.asarray(b_out, dtype=np.float32).reshape(KC, 128).T
    )

    # host-side enc/dec projections (0.3 of 17.5 GFLOP), fp32 then bf16
    enc = h_enc[:, :, 0, :] @ W_enc + b_enc      # (B, T, J)
    dec = h_dec[:, 0, :, :] @ W_dec              # (B, U, J)

    in_maps = []
    for c in range(NCORES):
        b, th = c // 2, c % 2
        encT = enc[b, th * TH:(th + 1) * TH, :].T          # (J, TH)
        encd = pmajor(
            np.repeat(encT, 2, axis=1).astype(bfl)
        )  # each value duplicated into adjacent pairs
        dec16 = pmajor(dec[b].T.astype(bfl))
        in_maps.append(
            {
                "encd": encd,
                "dec16": dec16,
                "wout": wout_b,
                "bout": bout_cols,
            }
        )

    global _last_in_maps
    _last_in_maps = in_maps
    res = run_bass_kernel_spmd(nc, in_maps, list(range(NCORES)))

    out_full = np.empty((B, T, U, V), dtype=np.float32)
    for c in range(NCORES):
        b, th = c // 2, c % 2
        oc = np.asarray(res.results[c]["out"]).astype(np.float32)  # (512, 8192)
        out_full[b, th * TH:(th + 1) * TH] = oc.reshape(V, TH, U).transpose(
            1, 2, 0
        )
    return out_full
